# revision 1
# baseline (speedup 1.0000x reference)
"""DeepseekV3Attention (MLA) forward, 8-way sharded Trainium2 Bass kernel.

Sharding: tensor-parallel over heads (16 heads -> 2 per core) for
q_b/kv_b/attention/o_proj; the low-rank a-path (q_a_proj, kv_a_proj,
RMSNorms, k-RoPE) is sequence-sharded (256 tokens per core) and the
latents are AllGathered (bf16).  o_proj is column-sharded after
per-block AllGathers of the attention outputs (so o_proj pipelines
under the attention of later blocks).

All device activations are feature-major ([features, tokens]); the
host pre-transposes hidden_states and all weights so no on-device
transposes are needed.  RoPE's deinterleave and rotate_half are folded
into host-side permutations of the rope weight columns (an extra
"shifted" output block per rope projection).
"""

import numpy as np
import ml_dtypes

import concourse.bacc as bacc
import concourse.mybir as mybir
from concourse.tile import TileContext
from concourse.bass_utils import run_bass_kernel_spmd

# ---- problem dims (hardcoded; must match the reference) ----
B, S, HID = 1, 2048, 2048
NH = 16
DN = 128          # qk_nope_head_dim
DR = 64           # qk_rope_head_dim
DH = DN + DR      # 192
DV = 128          # v_head_dim
QL = 1536         # q_lora_rank
KL = 512          # kv_lora_rank
EPS = 1e-6
_M = 0.1 * 1.0 * float(np.log(40.0)) + 1.0
SCALING = DH ** (-0.5) * _M * _M

NC = 8            # cores
HL = NH // NC     # 2 heads per core
SL = S // NC      # 256 tokens per core (a-path shard)
HIDL = HID // NC  # 256 output cols per core (o_proj shard)

F32 = mybir.dt.float32
BF16 = mybir.dt.bfloat16
BF = ml_dtypes.bfloat16

# agk rows (bf16 x SL): k_lat (KL) | k_rot roped (DR) | r_k fp32-bitcast (2)
AGK_ROWS = KL + DR + 2   # 578
# agq rows: q_lat (QL) | r_q fp32-bitcast (2)
AGQ_ROWS = QL + 2        # 1538

_ROT_PERM = np.array([2 * j for j in range(32)] + [2 * j + 1 for j in range(32)])
_SHIFT_PERM = np.array([(d + 32) % 64 for d in range(64)])

_BUILD_CACHE = {}


def _build_nc():
    if "nc" in _BUILD_CACHE:
        return _BUILD_CACHE["nc"]
    nc = bacc.Bacc(None, target_bir_lowering=False, num_devices=NC)

    # ---------------- DRAM I/O ----------------
    d_xT = nc.dram_tensor("xT", [HID, SL], BF16, kind="ExternalInput")
    d_wqaT = nc.dram_tensor("wqaT", [HID, QL], BF16, kind="ExternalInput")
    d_wkvaT = nc.dram_tensor("wkvaT", [HID, KL + 2 * DR], BF16, kind="ExternalInput")
    # q_b cols: [h0 pass 128 | h1 pass 128 | h0 rot 64 | h1 rot 64 | h0 rsh 64 | h1 rsh 64]
    d_wqbT = nc.dram_tensor("wqbT", [QL, 4 * 128], BF16, kind="ExternalInput")
    # kv_b cols: [h0 kpass 128 | h1 kpass 128 | h0 v 128 | h1 v 128]
    d_wkvbT = nc.dram_tensor("wkvbT", [KL, 4 * 128], BF16, kind="ExternalInput")
    d_woT = nc.dram_tensor("woT", [NH * DV, HIDL], BF16, kind="ExternalInput")
    d_cosS = nc.dram_tensor("cosS", [DR, SL], F32, kind="ExternalInput")
    d_sinS = nc.dram_tensor("sinS", [DR, SL], F32, kind="ExternalInput")  # signed
    d_cosF = nc.dram_tensor("cosF", [2 * DR, S], BF16, kind="ExternalInput")
    d_sinF = nc.dram_tensor("sinF", [2 * DR, S], BF16, kind="ExternalInput")  # signed

    d_out = nc.dram_tensor("o", [HIDL, S], F32, kind="ExternalOutput")

    agk_in = nc.dram_tensor("agk_in", [AGK_ROWS, SL], BF16)
    agk_out = nc.dram_tensor("agk_out", [NC * AGK_ROWS, SL], BF16, addr_space="Shared")
    AGQ1_ROWS = 768
    AGQ2_ROWS = QL - 768 + 2
    agq1_in = nc.dram_tensor("agq1_in", [AGQ1_ROWS, SL], BF16)
    agq1_out = nc.dram_tensor("agq1_out", [NC * AGQ1_ROWS, SL], BF16,
                              addr_space="Shared")
    agq2_in = nc.dram_tensor("agq2_in", [AGQ2_ROWS, SL], BF16)
    agq2_out = nc.dram_tensor("agq2_out", [NC * AGQ2_ROWS, SL], BF16,
                              addr_space="Shared")
    ag2_in = [nc.dram_tensor(f"ag2_in{b}", [HL * DV, 512], BF16) for b in range(4)]
    ag2_out = [nc.dram_tensor(f"ag2_out{b}", [NH * DV, 512], BF16,
                              addr_space="Shared") for b in range(4)]
    # last block gathered per head-half so o_proj(3) starts on the first half
    ag3h_in = [nc.dram_tensor(f"ag3h_in{h}", [DV, 512], BF16) for h in range(2)]
    ag3h_out = [nc.dram_tensor(f"ag3h_out{h}", [NC * DV, 512], BF16,
                               addr_space="Shared") for h in range(2)]

    cc_sem = nc.alloc_semaphore("cc_sem")
    RG = [list(range(NC))]
    ccv = [0]  # collective completion counter

    with TileContext(nc) as tc:

        def collective(in_t, out_t):
            with tc.tile_critical():
                ccv[0] += 1
                nc.gpsimd.collective_compute(
                    "AllGather", mybir.AluOpType.bypass, replica_groups=RG,
                    ins=[in_t[:]], outs=[out_t[:]],
                ).then_inc(cc_sem, 1)
                nc.gpsimd.wait_ge(cc_sem, ccv[0])

        with tc.tile_pool(name="const", bufs=1) as cpool:
            ones_f32 = cpool.tile([128, 1], F32, tag="ones_f32")
            nc.vector.memset(ones_f32[:], 1.0)
            ones1_f32 = cpool.tile([1, 128], F32, tag="ones1_f32")
            nc.vector.memset(ones1_f32[:], 1.0)
            ones_bf = cpool.tile([128, 1], BF16, tag="ones_bf")
            nc.vector.memset(ones_bf[:], 1.0)
            eps_t = cpool.tile([1, 1], F32, tag="eps")
            nc.vector.memset(eps_t[:], EPS)
            # causal diag mask: mask[p, j] = 1 if j >= p else 0
            mask128 = cpool.tile([128, 128], BF16, tag="mask128")
            nc.gpsimd.memset(mask128[:], 1.0)
            nc.gpsimd.affine_select(mask128[:], mask128[:], pattern=[[1, 128]],
                                    compare_op=mybir.AluOpType.is_ge,
                                    fill=0.0, base=0, channel_multiplier=-1)
            # stage-C/F weights: tiles allocated here (outermost pool), DMAs
            # emitted after stage A's wqa loads so the ACT hwdge queue serves
            # wqa first
            wo = cpool.tile([128, 16 * HIDL], BF16, tag="wo")
            wkvb = cpool.tile([128, 4 * 512], BF16, tag="wkvb")
            wqb = cpool.tile([128, 12 * 512], BF16, tag="wqb")
            cosF = cpool.tile([2 * DR, S], BF16, tag="cosF")
            sinF = cpool.tile([2 * DR, S], BF16, tag="sinF")

            # =========== Stage A + k-side stage B ===========
            # kpool (outermost): k-side tensors consumed by attention; opened
            # before the AGK/AGQ criticals so the k-side projections are not
            # serialized behind the AGQ collective.
            with tc.tile_pool(name="kpool", bufs=1) as kpool, \
                 tc.tile_pool(name="kwork", bufs=2) as kwork:
                Rk = kpool.tile([128, S], F32, tag="Rk")
                knorm = kpool.tile([128, 4 * S], BF16, tag="knorm")
                krot = kpool.tile([2 * DR, S], BF16, tag="krot")
                kpass = [kpool.tile([128, S], BF16, tag=f"kpass{h}", name=f"kpass{h}")
                         for h in range(HL)]
                vtok = kpool.tile([128, 16 * 256], BF16, tag="vtok")
                rr = kpool.tile([1, S], F32, tag="rr")

                with tc.tile_pool(name="stageA", bufs=1) as apool, \
                     tc.tile_pool(name="apsum", bufs=4, space="PSUM") as apsum, \
                     tc.tile_pool(name="awork", bufs=3) as awork:
                    WKW = KL + 2 * DR  # 640
                    xT = apool.tile([128, 16 * SL], BF16, tag="xT")
                    for g in range(4):
                        nc.sync.dma_start(
                            out=xT[:, g * 4 * SL:(g + 1) * 4 * SL],
                            in_=d_xT[g * 512:(g + 1) * 512, :]
                                .rearrange("(c p) q -> p c q", p=128))
                    wkva = apool.tile([128, 16 * WKW], BF16, tag="wkva")
                    for g in range(2):
                        nc.sync.dma_start(
                            out=wkva[:, g * 8 * WKW:(g + 1) * 8 * WKW],
                            in_=d_wkvaT[g * 1024:(g + 1) * 1024, :]
                                .rearrange("(c p) q -> p c q", p=128))
                    cosS = apool.tile([DR, SL], F32, tag="cosS")
                    nc.scalar.dma_start(out=cosS[:], in_=d_cosS[:])
                    sinS = apool.tile([DR, SL], F32, tag="sinS")
                    nc.scalar.dma_start(out=sinS[:], in_=d_sinS[:])
                    wqa = apool.tile([128, 16 * QL], BF16, tag="wqa")
                    for g in range(4):
                        nc.scalar.dma_start(
                            out=wqa[:, g * 4 * QL:(g + 1) * 4 * QL],
                            in_=d_wqaT[g * 512:(g + 1) * 512, :]
                                .rearrange("(c p) q -> p c q", p=128))
                    nc.sync.dma_start(
                        out=wkvb[:], in_=d_wkvbT.rearrange("(c p) q -> p c q", p=128))
                    nc.sync.dma_start(
                        out=wqb[:], in_=d_wqbT.rearrange("(c p) q -> p c q", p=128))
                    nc.sync.dma_start(out=cosF[:], in_=d_cosF[:])
                    nc.sync.dma_start(out=sinF[:], in_=d_sinF[:])
                    nc.sync.dma_start(
                        out=wo[:], in_=d_woT.rearrange("(c p) q -> p c q", p=128))

                    # ---- ckv path first (feeds AGK) ----
                    kstage = apool.tile([128, 4 * SL], BF16, tag="kstage")
                    ss_k = apsum.tile([1, SL], F32, tag="ps")
                    for m in range(4):
                        ps = apsum.tile([128, SL], F32, tag="ps")
                        for c in range(16):
                            nc.tensor.matmul(
                                ps[:],
                                wkva[:, c * WKW + m * 128: c * WKW + (m + 1) * 128],
                                xT[:, c * SL:(c + 1) * SL],
                                start=(c == 0), stop=(c == 15))
                        nc.vector.tensor_copy(kstage[:, m * SL:(m + 1) * SL], ps[:])
                        sq = awork.tile([128, SL], BF16, tag="a_sq")
                        nc.gpsimd.tensor_mul(sq[:], kstage[:, m * SL:(m + 1) * SL],
                                             kstage[:, m * SL:(m + 1) * SL])
                        nc.tensor.matmul(ss_k[:], ones_bf[:], sq[:],
                                         start=(m == 0), stop=(m == 3))
                    nc.gpsimd.dma_start(
                        out=agk_in[0:KL, :].rearrange("(c p) q -> p c q", p=128),
                        in_=kstage[:])
                    rk = awork.tile([1, SL], F32, tag="rq")
                    nc.scalar.activation(rk[:], ss_k[:],
                                         mybir.ActivationFunctionType.Sqrt,
                                         bias=eps_t[:], scale=1.0 / KL)
                    rki = awork.tile([1, SL], F32, tag="rqi")
                    nc.vector.reciprocal(rki[:], rk[:])
                    nc.gpsimd.dma_start(
                        out=agk_in[KL + DR: KL + DR + 2, :].rearrange("a b -> (a b)")
                            .bitcast(F32).rearrange("(a b) -> a b", a=1),
                        in_=rki[:])

                    ps_rot = apsum.tile([DR, SL], F32, tag="ps")
                    ps_rsh = apsum.tile([DR, SL], F32, tag="ps")
                    for c in range(16):
                        nc.tensor.matmul(ps_rot[:],
                                         wkva[:, c * WKW + KL: c * WKW + KL + DR],
                                         xT[:, c * SL:(c + 1) * SL],
                                         start=(c == 0), stop=(c == 15))
                    for c in range(16):
                        nc.tensor.matmul(ps_rsh[:],
                                         wkva[:, c * WKW + KL + DR: c * WKW + KL + 2 * DR],
                                         xT[:, c * SL:(c + 1) * SL],
                                         start=(c == 0), stop=(c == 15))
                    m1 = awork.tile([DR, SL], F32, tag="ro1")
                    nc.vector.tensor_mul(m1[:], ps_rot[:], cosS[:])
                    m2 = awork.tile([DR, SL], F32, tag="ro2")
                    nc.vector.tensor_mul(m2[:], ps_rsh[:], sinS[:])
                    kro = awork.tile([DR, SL], BF16, tag="kro")
                    nc.vector.tensor_add(kro[:], m1[:], m2[:])
                    nc.gpsimd.dma_start(out=agk_in[KL: KL + DR, :], in_=kro[:])

                    collective(agk_in, agk_out)

                    # ---- q_lat path (feeds AGQ1 + AGQ2) ----
                    qstage = apool.tile([128, 12 * SL], BF16, tag="qstage")
                    ss_q = apsum.tile([1, SL], F32, tag="ps")
                    for m in range(12):
                        ps = apsum.tile([128, SL], F32, tag="ps")
                        for c in range(16):
                            nc.tensor.matmul(
                                ps[:],
                                wqa[:, c * QL + m * 128: c * QL + (m + 1) * 128],
                                xT[:, c * SL:(c + 1) * SL],
                                start=(c == 0), stop=(c == 15))
                        nc.vector.tensor_copy(qstage[:, m * SL:(m + 1) * SL], ps[:])
                        sq = awork.tile([128, SL], BF16, tag="a_sq")
                        nc.gpsimd.tensor_mul(sq[:], qstage[:, m * SL:(m + 1) * SL],
                                             qstage[:, m * SL:(m + 1) * SL])
                        nc.tensor.matmul(ss_q[:], ones_bf[:], sq[:],
                                         start=(m == 0), stop=(m == 11))
                        if m == 5:
                            nc.gpsimd.dma_start(
                                out=agq1_in[:].rearrange("(c p) q -> p c q", p=128),
                                in_=qstage[:, 0:6 * SL])
                    nc.gpsimd.dma_start(
                        out=agq2_in[0:768, :].rearrange("(c p) q -> p c q", p=128),
                        in_=qstage[:, 6 * SL:12 * SL])
                    rq = awork.tile([1, SL], F32, tag="rq")
                    nc.scalar.activation(rq[:], ss_q[:],
                                         mybir.ActivationFunctionType.Sqrt,
                                         bias=eps_t[:], scale=1.0 / QL)
                    rqi = awork.tile([1, SL], F32, tag="rqi")
                    nc.vector.reciprocal(rqi[:], rq[:])
                    nc.gpsimd.dma_start(
                        out=agq2_in[768: 770, :].rearrange("a b -> (a b)")
                            .bitcast(F32).rearrange("(a b) -> a b", a=1),
                        in_=rqi[:])

                    collective(agq1_in, agq1_out)
                    collective(agq2_in, agq2_out)

                    # ---- k-side stage B (depends only on AGK) ----
                    agk_v = agk_out.rearrange("(r a) b -> a r b", r=NC)
                    nc.scalar.dma_start(
                        out=rr[:],
                        in_=agk_v[KL + DR: KL + DR + 2]
                            .rearrange("a r b -> r (a b)").bitcast(F32))
                    for b in range(4):
                        ps = apsum.tile([128, 512], F32, tag="kps", bufs=2)
                        nc.tensor.matmul(ps[:], ones1_f32[:],
                                         rr[:, b * 512:(b + 1) * 512],
                                         start=True, stop=True)
                        nc.vector.tensor_copy(Rk[:, b * 512:(b + 1) * 512], ps[:])
                    for m in range(4):
                        kraw = kwork.tile([128, S], BF16, tag="kraw")
                        nc.sync.dma_start(out=kraw[:], in_=agk_v[m * 128:(m + 1) * 128])
                        for b in range(4):
                            nc.vector.tensor_mul(
                                knorm[:, m * S + b * 512: m * S + (b + 1) * 512],
                                kraw[:, b * 512:(b + 1) * 512],
                                Rk[:, b * 512:(b + 1) * 512])
                    for half in range(2):
                        nc.scalar.dma_start(out=krot[half * DR:(half + 1) * DR, :],
                                            in_=agk_v[KL: KL + DR])
                    for h in range(HL):
                        for b in range(4):
                            ps = apsum.tile([128, 512], F32, tag="kps", bufs=2)
                            for c in range(4):
                                nc.tensor.matmul(
                                    ps[:],
                                    wkvb[:, c * 512 + h * 128: c * 512 + (h + 1) * 128],
                                    knorm[:, c * S + b * 512: c * S + (b + 1) * 512],
                                    start=(c == 0), stop=(c == 3))
                            nc.vector.tensor_copy(kpass[h][:, b * 512:(b + 1) * 512],
                                                  ps[:])
                    for sc in range(16):
                        ps = apsum.tile([128, 256], F32, tag="kps", bufs=2)
                        for c in range(4):
                            nc.tensor.matmul(
                                ps[:],
                                knorm[:, c * S + sc * 128: c * S + (sc + 1) * 128],
                                wkvb[:, c * 512 + 256: c * 512 + 512],
                                start=(c == 0), stop=(c == 3))
                        nc.vector.tensor_copy(vtok[:, sc * 256:(sc + 1) * 256], ps[:])

                # =========== q-side stages C/D/F ===========
                agq1_v = agq1_out.rearrange("(r a) b -> a r b", r=NC)
                agq2_v = agq2_out.rearrange("(r a) b -> a r b", r=NC)

                with tc.tile_pool(name="bigq", bufs=1) as bigq, \
                     tc.tile_pool(name="bpsum", bufs=6, space="PSUM") as bpsum, \
                     tc.tile_pool(name="fpool", bufs=2) as fpool, \
                     tc.tile_pool(name="bwork", bufs=3) as bwork:

                    # ---- R_q broadcast ----
                    Rq = bigq.tile([128, S], F32, tag="Rq")
                    rrq = bwork.tile([1, S], F32, tag="rr2", bufs=1, name="rrq")
                    nc.sync.dma_start(
                        out=rrq[:],
                        in_=agq2_v[768: 770]
                            .rearrange("a r b -> r (a b)").bitcast(F32))
                    for b in range(4):
                        ps = bpsum.tile([128, 512], F32, tag="fps", bufs=2, name="ps")
                        nc.tensor.matmul(ps[:], ones1_f32[:],
                                         rrq[:, b * 512:(b + 1) * 512],
                                         start=True, stop=True)
                        nc.vector.tensor_copy(Rq[:, b * 512:(b + 1) * 512], ps[:])

                    # ---- q_b + rope + Rq ----
                    qpass = [bigq.tile([128, S], BF16, tag=f"qpass{h}",
                                       name=f"qpass{h}") for h in range(HL)]
                    qrot = bigq.tile([128, S], BF16, tag="qrot")  # h0 64 | h1 64
                    for b in range(4):
                        sl = slice(b * 512, (b + 1) * 512)
                        ps_p = [bpsum.tile([128, 512], F32, tag="ps", name=f"ps_p{_i}")
                                for _i in range(2)]
                        ps_r = bpsum.tile([128, 512], F32, tag="ps")
                        ps_rs = bpsum.tile([128, 512], F32, tag="ps")
                        for c in range(12):
                            ql_t = bwork.tile([128, 512], BF16, tag="ql", bufs=8,
                                              name="ql_t")
                            src_v = agq1_v if c < 6 else agq2_v
                            cc0 = c * 128 if c < 6 else (c - 6) * 128
                            nc.sync.dma_start(
                                out=ql_t[:],
                                in_=src_v[cc0:cc0 + 128, 2 * b:2 * b + 2, :])
                            st, sp = (c == 0), (c == 11)
                            nc.tensor.matmul(ps_p[0][:],
                                             wqb[:, c * 512 + 0:c * 512 + 128],
                                             ql_t[:], start=st, stop=sp)
                            nc.tensor.matmul(ps_p[1][:],
                                             wqb[:, c * 512 + 128:c * 512 + 256],
                                             ql_t[:], start=st, stop=sp)
                            nc.tensor.matmul(ps_r[:],
                                             wqb[:, c * 512 + 256:c * 512 + 384],
                                             ql_t[:], start=st, stop=sp)
                            nc.tensor.matmul(ps_rs[:],
                                             wqb[:, c * 512 + 384:c * 512 + 512],
                                             ql_t[:], start=st, stop=sp)
                        for h in range(HL):
                            nc.vector.tensor_mul(qpass[h][:, sl], ps_p[h][:], Rq[:, sl])
                        t1 = bwork.tile([128, 512], F32, tag="ro1", bufs=2)
                        nc.vector.tensor_mul(t1[:], ps_r[:], cosF[:, sl])
                        t2 = bwork.tile([128, 512], F32, tag="ro2", bufs=2)
                        nc.vector.tensor_mul(t2[:], ps_rs[:], sinF[:, sl])
                        t3 = bwork.tile([128, 512], F32, tag="ro3", bufs=2)
                        nc.vector.tensor_add(t3[:], t1[:], t2[:])
                        nc.vector.tensor_mul(qrot[:, sl], t3[:], Rq[:, sl])

                    # ---- attention + per-block AG2 + o_proj ----
                    def oproj(b):
                        qsl = slice(b * 512, (b + 1) * 512)
                        collective(ag2_in[b], ag2_out[b])
                        octx = fpool.tile([128, 16 * 512], BF16, tag="octx",
                                          name="octx")
                        for g in range(4):
                            nc.sync.dma_start(
                                out=octx[:, g * 4 * 512:(g + 1) * 4 * 512],
                                in_=ag2_out[b][g * 512:(g + 1) * 512, :]
                                    .rearrange("(c p) q -> p c q", p=128))
                        for m in range(2):
                            ps = bpsum.tile([128, 512], F32, tag="fps", bufs=2,
                                            name="fps")
                            for c in range(16):
                                nc.tensor.matmul(
                                    ps[:],
                                    wo[:, c * HIDL + m * 128: c * HIDL + (m + 1) * 128],
                                    octx[:, c * 512:(c + 1) * 512],
                                    start=(c == 0), stop=(c == 15))
                            fo = fpool.tile([128, 512], F32, tag="fo", name="fo")
                            nc.vector.tensor_copy(fo[:], ps[:])
                            nc.gpsimd.dma_start(out=d_out[m * 128:(m + 1) * 128, qsl],
                                                in_=fo[:])

                    for b in range(4):
                        qsl = slice(b * 512, (b + 1) * 512)
                        for h in range(HL):
                            z_ps = bpsum.tile([1, 512], F32, tag="ps")
                            o_ps = bpsum.tile([128, 512], F32, tag="ps")
                            ncc = 4 * b + 4
                            for c in range(ncc):
                                s_ps = bpsum.tile([128, 512], F32, tag="ps")
                                nc.tensor.matmul(s_ps[:],
                                                 kpass[h][:, c * 128:(c + 1) * 128],
                                                 qpass[h][:, qsl],
                                                 start=True, stop=False)
                                nc.tensor.matmul(
                                    s_ps[:],
                                    krot[h * DR:(h + 1) * DR, c * 128:(c + 1) * 128],
                                    qrot[h * 64:(h + 1) * 64, qsl],
                                    start=False, stop=True)
                                E = bwork.tile([128, 512], BF16, tag="E")
                                nc.scalar.activation(E[:], s_ps[:],
                                                     mybir.ActivationFunctionType.Exp)
                                for j in range(4):
                                    base = 512 * b + 128 * j - 128 * c
                                    if base >= 128:
                                        continue
                                    elif base <= -128:
                                        nc.vector.memset(E[:, 128 * j:128 * (j + 1)],
                                                         0.0)
                                    else:
                                        nc.vector.tensor_mul(
                                            E[:, 128 * j:128 * (j + 1)],
                                            E[:, 128 * j:128 * (j + 1)],
                                            mask128[:])
                                nc.tensor.matmul(z_ps[:], ones_bf[:], E[:],
                                                 start=(c == 0), stop=(c == ncc - 1))
                                nc.tensor.matmul(
                                    o_ps[:],
                                    vtok[:, c * 256 + h * 128: c * 256 + (h + 1) * 128],
                                    E[:], start=(c == 0), stop=(c == ncc - 1))
                            zi = bwork.tile([1, 512], F32, tag="zi", bufs=2)
                            nc.vector.reciprocal(zi[:], z_ps[:])
                            zb_ps = bpsum.tile([128, 512], F32, tag="ps")
                            nc.tensor.matmul(zb_ps[:], ones1_f32[:], zi[:],
                                             start=True, stop=True)
                            zb = bwork.tile([128, 512], F32, tag="zbs", bufs=2)
                            nc.vector.tensor_copy(zb[:], zb_ps[:])
                            on = bwork.tile([128, 512], BF16, tag="on")
                            nc.vector.tensor_mul(on[:], o_ps[:], zb[:])
                            if b == 3:
                                nc.gpsimd.dma_start(out=ag3h_in[h][:], in_=on[:])
                            else:
                                nc.gpsimd.dma_start(
                                    out=ag2_in[b][h * DV:(h + 1) * DV, :], in_=on[:])

                        # defer block b-1's AG2+o_proj until after block b's
                        # attention is emitted
                        if b >= 1:
                            oproj(b - 1)
                        if b == 3:
                            collective(ag3h_in[0], ag3h_out[0])
                            collective(ag3h_in[1], ag3h_out[1])
                    # o_proj for the last block: accumulate even-head rows
                    # (h0 halves) as soon as their gather lands, then odd
                    qsl = slice(3 * 512, 4 * 512)
                    octx = fpool.tile([128, 16 * 512], BF16, tag="octx",
                                      name="octx")
                    for hh in range(2):
                        for g in range(2):
                            nc.sync.dma_start(
                                out=octx[:, (hh * 2 + g) * 4 * 512:
                                         (hh * 2 + g + 1) * 4 * 512],
                                in_=ag3h_out[hh][g * 512:(g + 1) * 512, :]
                                    .rearrange("(c p) q -> p c q", p=128))
                    for m in range(2):
                        ps = bpsum.tile([128, 512], F32, tag="fps", bufs=2,
                                        name="fps")
                        mm = 0
                        for hh in range(2):
                            for r in range(NC):
                                c = 2 * r + hh   # global head index
                                nc.tensor.matmul(
                                    ps[:],
                                    wo[:, c * HIDL + m * 128: c * HIDL + (m + 1) * 128],
                                    octx[:, (hh * 8 + r) * 512:(hh * 8 + r + 1) * 512],
                                    start=(mm == 0), stop=(mm == 15))
                                mm += 1
                        fo = fpool.tile([128, 512], F32, tag="fo", name="fo")
                        nc.vector.tensor_copy(fo[:], ps[:])
                        nc.gpsimd.dma_start(out=d_out[m * 128:(m + 1) * 128, qsl],
                                            in_=fo[:])

    nc.finalize()
    _BUILD_CACHE["nc"] = nc
    return nc


def _prep_inputs(hidden_states, cos, sin, w_qa, w_qa_ln, w_qb, w_kva, w_kva_ln,
                 w_kvb, w_o):
    """Host-side shard + transpose + fold.  Returns per-core in_maps."""
    f32 = np.float32
    X = np.asarray(hidden_states, f32).reshape(S, HID)
    XT = np.ascontiguousarray(X.T)                      # [HID, S]
    w_qa = np.asarray(w_qa, f32)
    w_qb = np.asarray(w_qb, f32)
    w_kva = np.asarray(w_kva, f32)
    w_kvb = np.asarray(w_kvb, f32)
    w_o = np.asarray(w_o, f32)
    w_qa_ln = np.asarray(w_qa_ln, f32)
    w_kva_ln = np.asarray(w_kva_ln, f32)
    cos = np.asarray(cos, f32).reshape(S, DR)
    sin = np.asarray(sin, f32).reshape(S, DR)

    wqaT = np.ascontiguousarray(w_qa.T).astype(BF)       # [HID, QL]

    wkvaT = w_kva.T                                      # [HID, KL+DR]
    rot = wkvaT[:, KL:][:, _ROT_PERM]                    # deinterleaved
    rotsh = rot[:, _SHIFT_PERM]
    wkvaT_ext = np.ascontiguousarray(
        np.concatenate([wkvaT[:, :KL], rot, rotsh], axis=1)).astype(BF)

    wqbT = (w_qb * w_qa_ln[None, :]).T * SCALING         # [QL, NH*DH]
    wqbT = wqbT.reshape(QL, NH, DH)
    qb_pass = wqbT[:, :, :DN]
    qb_rot = wqbT[:, :, DN:][:, :, _ROT_PERM]
    qb_rotsh = qb_rot[:, :, _SHIFT_PERM]

    wkvbT = (w_kvb * w_kva_ln[None, :]).T                # [KL, NH*(DN+DV)]
    wkvbT = wkvbT.reshape(KL, NH, DN + DV)
    kvb_k = wkvbT[:, :, :DN]
    kvb_v = wkvbT[:, :, DN:]

    woT = np.ascontiguousarray(w_o.T).astype(BF)         # [NH*DV, HID]

    cosT = np.ascontiguousarray(cos.T)                   # [DR, S]
    sinT = np.ascontiguousarray(sin.T)
    sin_signed = sinT.copy()
    sin_signed[:DR // 2] *= -1.0
    cosF = np.ascontiguousarray(np.concatenate([cosT, cosT], axis=0)).astype(BF)
    sinF = np.ascontiguousarray(
        np.concatenate([sin_signed, sin_signed], axis=0)).astype(BF)

    in_maps = []
    for i in range(NC):
        h0, h1 = HL * i, HL * i + 1
        wqbT_i = np.concatenate([
            qb_pass[:, h0], qb_pass[:, h1],
            qb_rot[:, h0], qb_rot[:, h1],
            qb_rotsh[:, h0], qb_rotsh[:, h1]], axis=1).astype(BF)
        wkvbT_i = np.concatenate([
            kvb_k[:, h0], kvb_k[:, h1],
            kvb_v[:, h0], kvb_v[:, h1]], axis=1).astype(BF)
        sls = slice(SL * i, SL * (i + 1))
        in_maps.append(dict(
            xT=np.ascontiguousarray(XT[:, sls]).astype(BF),
            wqaT=wqaT,
            wkvaT=wkvaT_ext,
            wqbT=np.ascontiguousarray(wqbT_i),
            wkvbT=np.ascontiguousarray(wkvbT_i),
            woT=np.ascontiguousarray(woT[:, HIDL * i: HIDL * (i + 1)]),
            cosS=np.ascontiguousarray(cosT[:, sls]),
            sinS=np.ascontiguousarray(sin_signed[:, sls]),
            cosF=cosF,
            sinF=sinF,
        ))
    return in_maps


def kernel(**inputs):
    nc = _build_nc()
    in_maps = _prep_inputs(**inputs)
    res = run_bass_kernel_spmd(nc, in_maps, core_ids=list(range(NC)))
    finalT = np.concatenate([res.results[i]["o"] for i in range(NC)], axis=0)
    out = np.ascontiguousarray(finalT.T).reshape(B, S, HID).astype(np.float32)
    return out



# revision 17
# speedup vs baseline: 1.1382x; 1.1382x over previous
"""DeepseekV3Attention (MLA) forward, 8-way sharded Trainium2 Bass kernel.

Sharding: tensor-parallel over heads (16 heads -> 2 per core) for
q_b/kv_b/attention/o_proj; the low-rank a-path (q_a_proj, kv_a_proj,
RMSNorms, k-RoPE) is sequence-sharded (256 tokens per core) and the
latents are AllGathered pre-scaled by their RMSNorm factors (so no
post-gather normalization is needed).  o_proj is column-sharded after
per-block AllGathers of the attention outputs.

Device layout is feature-major ([features, tokens]) throughout.  RoPE's
rotate-half is a 128x128 permutation matmul on the PE (signs folded
into the host-side sin table).

Attention is mixed precision: key-chunks far below the causal diagonal
(cc < 4b for query block b) run scores and exp in fp8 with DoubleRow
perf mode (pass|rope contraction pair per score matmul, chunk pairs
for the softmax-sum); the four diagonal-adjacent chunks per block run
in bf16 with triangle-trimmed matmuls.  v stays bf16 (stationary
operand, so it never limits matmul rate).
"""

import numpy as np
import ml_dtypes

import concourse.bacc as bacc
import concourse.mybir as mybir
from concourse.tile import TileContext
from concourse.bass_utils import run_bass_kernel_spmd

# ---- problem dims (hardcoded; must match the reference) ----
B, S, HID = 1, 2048, 2048
NH = 16
DN = 128          # qk_nope_head_dim
DR = 64           # qk_rope_head_dim
DH = DN + DR      # 192
DV = 128          # v_head_dim
QL = 1536         # q_lora_rank
KL = 512          # kv_lora_rank
EPS = 1e-6
_M = 0.1 * 1.0 * float(np.log(40.0)) + 1.0
SCALING = DH ** (-0.5) * _M * _M

NC = 8            # cores
HL = NH // NC     # 2 heads per core
SL = S // NC      # 256 tokens per core (a-path shard)
HIDL = HID // NC  # 256 output cols per core (o_proj shard)
WKW = KL + DR     # kv_a cols: 512 latent + 64 rope (deinterleaved)

F32 = mybir.dt.float32
BF16 = mybir.dt.bfloat16
FP8 = mybir.dt.float8e4
BF = ml_dtypes.bfloat16
DRM = mybir.MatmulPerfMode.DoubleRow
AF = mybir.ActivationFunctionType

# AGK rows (bf16 x SL):
#   scaled k latents (KL) | roped k bf16 (DR) | roped k fp8 (DR as DR/2 bf16)
AGK_ROWS = KL + DR + DR // 2   # 608

_ROT_PERM = np.array([2 * j for j in range(32)] + [2 * j + 1 for j in range(32)])

_BUILD_CACHE = {}


def _build_nc():
    if "nc" in _BUILD_CACHE:
        return _BUILD_CACHE["nc"]
    nc = bacc.Bacc(None, target_bir_lowering=False, num_devices=NC)

    # ---------------- DRAM I/O ----------------
    d_xT = nc.dram_tensor("xT", [HID, SL], BF16, kind="ExternalInput")
    d_wqaT = nc.dram_tensor("wqaT", [HID, QL], BF16, kind="ExternalInput")
    d_wkvaT = nc.dram_tensor("wkvaT", [HID, WKW], BF16, kind="ExternalInput")
    # q_b cols: [h0 pass 128 | h1 pass 128 | rot 128 (h0 64 | h1 64)]
    d_wqbT = nc.dram_tensor("wqbT", [QL, 3 * 128], BF16, kind="ExternalInput")
    # kv_b cols: [h0 kpass 128 | h1 kpass 128 | h0 v 128 | h1 v 128]
    d_wkvbT = nc.dram_tensor("wkvbT", [KL, 4 * 128], BF16, kind="ExternalInput")
    d_woT = nc.dram_tensor("woT", [NH * DV, HIDL], BF16, kind="ExternalInput")
    d_cosS = nc.dram_tensor("cosS", [DR, SL], F32, kind="ExternalInput")
    d_sinS = nc.dram_tensor("sinS", [DR, SL], F32, kind="ExternalInput")  # signed
    d_cosQ = nc.dram_tensor("cosQ", [2 * DR, S], BF16, kind="ExternalInput")
    d_sinQ = nc.dram_tensor("sinQ", [2 * DR, S], BF16, kind="ExternalInput")
    d_perm = nc.dram_tensor("permM", [128, 128], BF16, kind="ExternalInput")

    d_out = nc.dram_tensor("o", [HIDL, S], F32, kind="ExternalOutput")

    agk_in = nc.dram_tensor("agk_in", [AGK_ROWS, SL], BF16)
    agk_out = nc.dram_tensor("agk_out", [NC * AGK_ROWS, SL], BF16,
                             addr_space="Shared")
    agq_in = nc.dram_tensor("agq_in", [QL, SL], BF16)
    agq_out = nc.dram_tensor("agq_out", [NC * QL, SL], BF16, addr_space="Shared")
    ag2_in = [nc.dram_tensor(f"ag2_in{b}", [HL * DV, 512], BF16) for b in range(3)]
    ag2_out = [nc.dram_tensor(f"ag2_out{b}", [NH * DV, 512], BF16,
                              addr_space="Shared") for b in range(3)]
    # last block gathered per head so o_proj(3) starts on the first head
    ag3h_in = [nc.dram_tensor(f"ag3h_in{h}", [DV, 512], BF16) for h in range(2)]
    ag3h_out = [nc.dram_tensor(f"ag3h_out{h}", [NC * DV, 512], BF16,
                               addr_space="Shared") for h in range(2)]

    cc_sem = nc.alloc_semaphore("cc_sem")
    RG = [list(range(NC))]
    ccv = [0]  # collective completion counter

    with TileContext(nc) as tc:

        def collective(in_t, out_t):
            with tc.tile_critical():
                ccv[0] += 1
                nc.gpsimd.collective_compute(
                    "AllGather", mybir.AluOpType.bypass, replica_groups=RG,
                    ins=[in_t[:]], outs=[out_t[:]],
                ).then_inc(cc_sem, 1)
                nc.gpsimd.wait_ge(cc_sem, ccv[0])

        with tc.tile_pool(name="const", bufs=1) as cpool:
            ones_bf = cpool.tile([128, 1], BF16, tag="ones_bf")
            nc.vector.memset(ones_bf[:], 1.0)
            ones8 = cpool.tile([128, 2, 32], FP8, tag="ones8")
            nc.vector.memset(ones8[:], 1.0)
            eps_t = cpool.tile([1, 1], F32, tag="eps")
            nc.vector.memset(eps_t[:], EPS)
            neg1 = cpool.tile([128, 1], F32, tag="neg1")
            nc.vector.memset(neg1[:], -1.0)
            # causal diag mask: mask[p, j] = 1 if j >= p else 0
            mask128 = cpool.tile([128, 128], BF16, tag="mask128")
            nc.gpsimd.memset(mask128[:], 1.0)
            nc.gpsimd.affine_select(mask128[:], mask128[:], pattern=[[1, 128]],
                                    compare_op=mybir.AluOpType.is_ge,
                                    fill=0.0, base=0, channel_multiplier=-1)
            # const-pool weight tiles; DMAs are emitted inside stage A after
            # the latency-critical xT/wkva/wqa loads so the SP queue serves
            # those first
            permM = cpool.tile([128, 128], BF16, tag="permM")
            cosQ = cpool.tile([2 * DR, S], BF16, tag="cosQ")
            sinQ = cpool.tile([2 * DR, S], BF16, tag="sinQ")
            wkvb = cpool.tile([128, 4, 512], BF16, tag="wkvb")
            wqb = cpool.tile([128, 12, 384], BF16, tag="wqb")
            wo = cpool.tile([128, 16, HIDL], BF16, tag="wo")

            # kpool: attention inputs, persistent through the whole kernel
            with tc.tile_pool(name="kpool", bufs=1) as kpool:
                # fp8 pair tiles for the far-chunk DoubleRow score matmuls:
                # K8[h] slot 0 = k_pass(h), slot 1 = roped k_rot at
                # partition h*64 (other 64 partitions zero, pairing with
                # the shared q-rot tile that stacks both heads).
                K8 = [kpool.tile([128, 2, S], FP8, tag=f"K8_{h}", name=f"K8_{h}")
                      for h in range(HL)]
                # Q8[h]: slot 0 = q_pass(h); slot 1 = q_rot (both heads).
                Q8 = [kpool.tile([128, 2, S], FP8, tag=f"Q8_{h}", name=f"Q8_{h}")
                      for h in range(HL)]
                # bf16 copies for the diagonal-adjacent chunks
                kpassb = [kpool.tile([128, S], BF16, tag=f"kpb{h}",
                                     name=f"kpassb{h}") for h in range(HL)]
                qpassb = [kpool.tile([128, S], BF16, tag=f"qpb{h}",
                                     name=f"qpassb{h}") for h in range(HL)]
                # krot duplicated in both 64-row halves so each head's
                # score matmul sees matching base partitions
                krotb = kpool.tile([2 * DR, S], BF16, tag="krotb")
                qrotb = kpool.tile([128, S], BF16, tag="qrotb")
                vtokb = kpool.tile([128, 16, 2 * DV], BF16, tag="vtokb")
                nc.vector.memset(K8[0][64:128, 1, :], 0.0)
                nc.vector.memset(K8[1][0:64, 1, :], 0.0)

                # =========== Stage A: a-path projections + collectives =====
                with tc.tile_pool(name="stageA", bufs=1) as apool, \
                     tc.tile_pool(name="awork", bufs=3) as awork:
                    xT = apool.tile([128, 16, SL], BF16, tag="xT")
                    wkva = apool.tile([128, 16, WKW], BF16, tag="wkva")
                    wqa = apool.tile([128, 16, QL], BF16, tag="wqa")
                    for c in range(16):
                        nc.sync.dma_start(out=xT[:, c, :],
                                          in_=d_xT[c * 128:(c + 1) * 128, :])
                        nc.gpsimd.dma_start(out=wkva[:, c, :],
                                            in_=d_wkvaT[c * 128:(c + 1) * 128, :])
                        nc.scalar.dma_start(out=wqa[:, c, :],
                                            in_=d_wqaT[c * 128:(c + 1) * 128, :])
                    cosS = apool.tile([DR, SL], F32, tag="cosS")
                    nc.sync.dma_start(out=cosS[:], in_=d_cosS[:])
                    sinS = apool.tile([DR, SL], F32, tag="sinS")
                    nc.sync.dma_start(out=sinS[:], in_=d_sinS[:])
                    nc.sync.dma_start(out=permM[:], in_=d_perm[:])
                    nc.sync.dma_start(out=cosQ[:], in_=d_cosQ[:])
                    nc.sync.dma_start(out=sinQ[:], in_=d_sinQ[:])
                    nc.sync.dma_start(
                        out=wkvb[:], in_=d_wkvbT.rearrange("(c p) q -> p c q", p=128))
                    nc.sync.dma_start(
                        out=wqb[:], in_=d_wqbT.rearrange("(c p) q -> p c q", p=128))
                    nc.sync.dma_start(
                        out=wo[:], in_=d_woT.rearrange("(c p) q -> p c q", p=128))

                    # ---- pass 1: ckv + k-rope (feeds AGK) ----
                    kstage = apool.tile([128, 4, SL], BF16, tag="kstage")
                    ksc = apool.tile([128, 4, SL], BF16, tag="ksc")
                    with tc.tile_pool(name="aps1", bufs=1, space="PSUM") as aps1:
                        kps = [aps1.tile([128, SL], F32, tag=f"kps{m}",
                                         name=f"kps{m}") for m in range(4)]
                        kps_rot = aps1.tile([DR, SL], F32, tag="krot",
                                            name="kps_rot")
                        ss_k = aps1.tile([1, SL], F32, tag="ssk", name="ss_k")
                        for c in range(16):
                            for m in range(4):
                                nc.tensor.matmul(
                                    kps[m][:], wkva[:, c, m * 128:(m + 1) * 128],
                                    xT[:, c, :], start=(c == 0), stop=(c == 15))
                            nc.tensor.matmul(
                                kps_rot[:], wkva[:, c, KL:KL + DR],
                                xT[:, c, :], start=(c == 0), stop=(c == 15))
                        for m in range(4):
                            nc.vector.tensor_copy(kstage[:, m, :], kps[m][:])
                            sq = awork.tile([128, SL], BF16, tag="sq")
                            nc.gpsimd.tensor_mul(sq[:], kstage[:, m, :],
                                                 kstage[:, m, :])
                            nc.tensor.matmul(ss_k[:], ones_bf[:], sq[:],
                                             start=(m == 0), stop=(m == 3))
                        rk = awork.tile([1, SL], F32, tag="rk", bufs=1)
                        nc.scalar.activation(rk[:], ss_k[:], AF.Sqrt,
                                             bias=eps_t[:], scale=1.0 / KL)
                        rki = awork.tile([1, SL], F32, tag="rki", bufs=1)
                        nc.vector.reciprocal(rki[:], rk[:])
                        rkiB = awork.tile([128, SL], F32, tag="rkiB", bufs=1)
                        nc.gpsimd.partition_broadcast(rkiB[:], rki[:])
                        for m in range(4):
                            nc.vector.tensor_mul(ksc[:, m, :], kstage[:, m, :],
                                                 rkiB[:])
                        nc.gpsimd.dma_start(
                            out=agk_in[0:KL, :].rearrange("(c p) q -> p c q", p=128),
                            in_=ksc[:])
                        # k rope: rot*cos + perm(rot)*sin_signed
                        rot_sk = awork.tile([DR, SL], BF16, tag="rot_sk", bufs=1)
                        nc.vector.tensor_copy(rot_sk[:], kps_rot[:])
                        kps_sh = aps1.tile([DR, SL], F32, tag="ksh", name="kps_sh")
                        nc.tensor.matmul(kps_sh[:], permM[0:DR, 0:DR], rot_sk[:],
                                         start=True, stop=True)
                        m1k = awork.tile([DR, SL], F32, tag="m1k", bufs=1)
                        nc.vector.tensor_mul(m1k[:], kps_rot[:], cosS[:])
                        m2k = awork.tile([DR, SL], F32, tag="m2k", bufs=1)
                        nc.vector.tensor_mul(m2k[:], kps_sh[:], sinS[:])
                        kro_b = awork.tile([DR, SL], BF16, tag="kro_b", bufs=1)
                        nc.vector.tensor_add(kro_b[:], m1k[:], m2k[:])
                        nc.gpsimd.dma_start(out=agk_in[KL:KL + DR, :], in_=kro_b[:])
                        kro8 = awork.tile([DR, SL], FP8, tag="kro8", bufs=1)
                        nc.vector.tensor_copy(kro8[:], kro_b[:])
                        nc.gpsimd.dma_start(
                            out=agk_in[KL + DR:AGK_ROWS, :]
                                .rearrange("a b -> (a b)")
                                .bitcast(FP8).rearrange("(a b) -> a b", a=DR),
                            in_=kro8[:])

                    collective(agk_in, agk_out)

                    # ---- pass 2: q_a (feeds AGQ) ----
                    qstage = apool.tile([128, 12, SL], BF16, tag="qstage")
                    qsc = apool.tile([128, 12, SL], BF16, tag="qsc")
                    with tc.tile_pool(name="aps2", bufs=1, space="PSUM") as aps2:
                        # PSUM tiles round up to whole banks: run the 12
                        # accumulators as two groups of 6 (+1 for ss_q)
                        ss_q = aps2.tile([1, SL], F32, tag="ssq", name="ss_q")
                        for g in range(2):
                            qps = [aps2.tile([128, SL], F32, tag=f"qps{j}",
                                             name=f"qps{g}_{j}") for j in range(6)]
                            for c in range(16):
                                for j in range(6):
                                    m = 6 * g + j
                                    nc.tensor.matmul(
                                        qps[j][:], wqa[:, c, m * 128:(m + 1) * 128],
                                        xT[:, c, :], start=(c == 0), stop=(c == 15))
                            for j in range(6):
                                m = 6 * g + j
                                nc.vector.tensor_copy(qstage[:, m, :], qps[j][:])
                                sq = awork.tile([128, SL], BF16, tag="sq")
                                nc.gpsimd.tensor_mul(sq[:], qstage[:, m, :],
                                                     qstage[:, m, :])
                                nc.tensor.matmul(ss_q[:], ones_bf[:], sq[:],
                                                 start=(m == 0), stop=(m == 11))
                        rq = awork.tile([1, SL], F32, tag="rk", bufs=1, name="rq")
                        nc.scalar.activation(rq[:], ss_q[:], AF.Sqrt,
                                             bias=eps_t[:], scale=1.0 / QL)
                        rqi = awork.tile([1, SL], F32, tag="rki", bufs=1,
                                         name="rqi")
                        nc.vector.reciprocal(rqi[:], rq[:])
                        rqiB = awork.tile([128, SL], F32, tag="rkiB", bufs=1,
                                          name="rqiB")
                        nc.gpsimd.partition_broadcast(rqiB[:], rqi[:])
                        for m in range(12):
                            nc.vector.tensor_mul(qsc[:, m, :], qstage[:, m, :],
                                                 rqiB[:])
                        nc.gpsimd.dma_start(
                            out=agq_in[:].rearrange("(c p) q -> p c q", p=128),
                            in_=qsc[:])

                    collective(agq_in, agq_out)

                    # ---- stage B: kv_b from gathered scaled latents ----
                    agk_v = agk_out.rearrange("(r a) b -> a r b", r=NC)
                    knorm = apool.tile([128, 4, S], BF16, tag="knorm")
                    for m in range(4):
                        q = nc.sync if m % 2 == 0 else nc.scalar
                        q.dma_start(out=knorm[:, m, :],
                                    in_=agk_v[m * 128:(m + 1) * 128])
                    for half in range(2):
                        nc.sync.dma_start(
                            out=krotb[half * DR:(half + 1) * DR, :],
                            in_=agk_v[KL:KL + DR])
                    for h in range(HL):
                        for r in range(NC):
                            nc.scalar.dma_start(
                                out=K8[h][h * DR:(h + 1) * DR, 1,
                                          r * SL:(r + 1) * SL],
                                in_=agk_out[r * AGK_ROWS + KL + DR:
                                            (r + 1) * AGK_ROWS, :]
                                    .rearrange("a b -> (a b)").bitcast(FP8)
                                    .rearrange("(a b) -> a b", a=DR))
                    with tc.tile_pool(name="bpsA", bufs=2, space="PSUM") as bpsA:
                        for h in range(HL):
                            for b in range(4):
                                ps = bpsA.tile([128, 512], F32, tag="kp")
                                for c in range(4):
                                    nc.tensor.matmul(
                                        ps[:], wkvb[:, c, h * 128:(h + 1) * 128],
                                        knorm[:, c, b * 512:(b + 1) * 512],
                                        start=(c == 0), stop=(c == 3))
                                nc.vector.tensor_copy(
                                    kpassb[h][:, b * 512:(b + 1) * 512], ps[:])
                                if b < 3:
                                    nc.scalar.activation(
                                        K8[h][:, 0, b * 512:(b + 1) * 512], ps[:],
                                        AF.Copy)
                        for sc in range(16):
                            ps = bpsA.tile([128, 2 * DV], F32, tag="vp")
                            for c in range(4):
                                nc.tensor.matmul(
                                    ps[:], knorm[:, c, sc * 128:(sc + 1) * 128],
                                    wkvb[:, c, 256:512],
                                    start=(c == 0), stop=(c == 3))
                            nc.vector.tensor_copy(vtokb[:, sc, :], ps[:])

                # =========== Stage C: q_b + rope ===========
                agq_v = agq_out.rearrange("(r a) b -> a r b", r=NC)
                with tc.tile_pool(name="cpsum", bufs=1, space="PSUM") as cpsum, \
                     tc.tile_pool(name="cwork", bufs=2) as cwork:
                    for b in range(4):
                        qsl = slice(b * 512, (b + 1) * 512)
                        p0 = cpsum.tile([128, 512], F32, tag="p0", bufs=2)
                        p1 = cpsum.tile([128, 512], F32, tag="p1", bufs=2)
                        pr = cpsum.tile([128, 512], F32, tag="pr", bufs=2)
                        for c in range(12):
                            ql_t = cwork.tile([128, 512], BF16, tag="ql", bufs=8,
                                              name="ql_t")
                            nc.sync.dma_start(
                                out=ql_t[:],
                                in_=agq_v[c * 128:(c + 1) * 128, 2 * b:2 * b + 2, :])
                            st, sp = (c == 0), (c == 11)
                            nc.tensor.matmul(p0[:], wqb[:, c, 0:128], ql_t[:],
                                             start=st, stop=sp)
                            nc.tensor.matmul(p1[:], wqb[:, c, 128:256], ql_t[:],
                                             start=st, stop=sp)
                            nc.tensor.matmul(pr[:], wqb[:, c, 256:384], ql_t[:],
                                             start=st, stop=sp)
                        nc.vector.tensor_copy(qpassb[0][:, qsl], p0[:])
                        nc.vector.tensor_copy(qpassb[1][:, qsl], p1[:])
                        if b >= 1:
                            nc.scalar.activation(Q8[0][:, 0, qsl], p0[:], AF.Copy)
                            nc.scalar.activation(Q8[1][:, 0, qsl], p1[:], AF.Copy)
                        rot_s = cwork.tile([128, 512], BF16, tag="rot_s")
                        nc.vector.tensor_copy(rot_s[:], pr[:])
                        psh = cpsum.tile([128, 512], F32, tag="psh", bufs=2)
                        nc.tensor.matmul(psh[:], permM[:], rot_s[:],
                                         start=True, stop=True)
                        m1 = cwork.tile([128, 512], F32, tag="m1")
                        nc.vector.tensor_mul(m1[:], pr[:], cosQ[:, qsl])
                        m2 = cwork.tile([128, 512], F32, tag="m2")
                        nc.vector.tensor_mul(m2[:], psh[:], sinQ[:, qsl])
                        nc.vector.tensor_add(qrotb[:, qsl], m1[:], m2[:])
                        if b >= 1:
                            nc.scalar.activation(Q8[0][:, 1, qsl], qrotb[:, qsl],
                                                 AF.Copy)
                            nc.scalar.activation(Q8[1][:, 1, qsl], qrotb[:, qsl],
                                                 AF.Copy)

                # =========== attention + per-block AG + o_proj ===========
                with tc.tile_pool(name="bpsum", bufs=1, space="PSUM") as bpsum, \
                     tc.tile_pool(name="fpool", bufs=2) as fpool, \
                     tc.tile_pool(name="bwork", bufs=3) as bwork:

                    def oproj(b):
                        qsl = slice(b * 512, (b + 1) * 512)
                        collective(ag2_in[b], ag2_out[b])
                        octx = fpool.tile([128, 16, 512], BF16, tag="octx",
                                          name="octx")
                        for g in range(4):
                            q = [nc.sync, nc.gpsimd, nc.sync, nc.gpsimd][g]
                            q.dma_start(
                                out=octx[:, 4 * g:4 * (g + 1), :],
                                in_=ag2_out[b][g * 512:(g + 1) * 512, :]
                                    .rearrange("(c p) q -> p c q", p=128))
                        for m in range(2):
                            ps = bpsum.tile([128, 512], F32, tag="fps", bufs=2,
                                            name="fps")
                            for c in range(16):
                                nc.tensor.matmul(
                                    ps[:], wo[:, c, m * 128:(m + 1) * 128],
                                    octx[:, c, :], start=(c == 0), stop=(c == 15))
                            fo = fpool.tile([128, 512], F32, tag="fo", name="fo")
                            nc.vector.tensor_copy(fo[:], ps[:])
                            nc.gpsimd.dma_start(out=d_out[m * 128:(m + 1) * 128, qsl],
                                                in_=fo[:])

                    for b in range(4):
                        if b >= 1:
                            oproj(b - 1)
                        qsl = slice(b * 512, (b + 1) * 512)
                        for h in range(HL):
                            hsl = slice(h * DV, (h + 1) * DV)
                            z_ps = bpsum.tile([32, 512], F32, tag="zps",
                                              name=f"zps{b}{h}")
                            o_ps = bpsum.tile([128, 512], F32, tag="ops",
                                              name=f"ops{b}{h}")
                            # --- far chunk pairs: fp8 DoubleRow ---
                            for p in range(2 * b):
                                s2 = bpsum.tile([128, 1024], F32, tag="s2",
                                                bufs=2, name="s2")
                                E2 = bwork.tile([128, 1024], FP8, tag="E2",
                                                bufs=4, name="E2")
                                for i in range(2):
                                    cc = 2 * p + i
                                    nc.tensor.matmul(
                                        s2[:, i * 512:(i + 1) * 512],
                                        K8[h][:, :, cc * 128:(cc + 1) * 128],
                                        Q8[h][:, :, qsl],
                                        start=True, stop=True, perf_mode=DRM)
                                nc.scalar.activation(E2[:], s2[:], AF.Exp,
                                                     scale=SCALING, bias=neg1[:])
                                nc.tensor.matmul(
                                    z_ps[:], ones8[:],
                                    E2[:].rearrange("p (two n) -> p two n", two=2),
                                    start=(p == 0), stop=False, perf_mode=DRM)
                                for i in range(2):
                                    cc = 2 * p + i
                                    nc.tensor.matmul(
                                        o_ps[:], vtokb[:, cc, hsl],
                                        E2[:, i * 512:(i + 1) * 512],
                                        start=(p == 0 and i == 0), stop=False)
                            # --- diagonal-adjacent chunks: bf16, trimmed ---
                            for d in range(4):
                                cc = 4 * b + d
                                qlo = 128 * d
                                W = 512 - qlo
                                i = d % 2
                                if i == 0:
                                    s2n = bpsum.tile([128, 1024], F32, tag="s2",
                                                     bufs=2, name="s2n")
                                    E2b = bwork.tile([128, 1024], BF16, tag="Eb",
                                                    bufs=3, name="E2b")
                                ssl = slice(i * 512, i * 512 + W)
                                nc.tensor.matmul(
                                    s2n[:, ssl],
                                    kpassb[h][:, cc * 128:(cc + 1) * 128],
                                    qpassb[h][:, b * 512 + qlo:(b + 1) * 512],
                                    start=True, stop=False)
                                nc.tensor.matmul(
                                    s2n[:, ssl],
                                    krotb[h * DR:(h + 1) * DR,
                                          cc * 128:(cc + 1) * 128],
                                    qrotb[h * DR:(h + 1) * DR,
                                          b * 512 + qlo:(b + 1) * 512],
                                    start=False, stop=True)
                                nc.scalar.activation(E2b[:, ssl], s2n[:, ssl],
                                                     AF.Exp, scale=SCALING,
                                                     bias=neg1[:])
                                nc.vector.tensor_mul(
                                    E2b[:, i * 512:i * 512 + 128],
                                    E2b[:, i * 512:i * 512 + 128], mask128[:])
                                first = (b == 0 and d == 0)
                                nc.tensor.matmul(
                                    z_ps[0:1, qlo:512], ones_bf[:], E2b[:, ssl],
                                    start=first, stop=(d == 3))
                                nc.tensor.matmul(
                                    o_ps[:, qlo:512], vtokb[:, cc, hsl],
                                    E2b[:, ssl],
                                    start=first, stop=(d == 3))
                            zi = bwork.tile([1, 512], F32, tag="zi", bufs=2)
                            nc.vector.reciprocal(zi[:], z_ps[0:1, :])
                            zb = bwork.tile([128, 512], F32, tag="zb", bufs=2)
                            nc.gpsimd.partition_broadcast(zb[:], zi[:])
                            on = bwork.tile([128, 512], BF16, tag="on")
                            nc.vector.tensor_mul(on[:], o_ps[:], zb[:])
                            if b == 3:
                                nc.gpsimd.dma_start(out=ag3h_in[h][:], in_=on[:])
                                collective(ag3h_in[h], ag3h_out[h])
                            else:
                                nc.gpsimd.dma_start(
                                    out=ag2_in[b][h * DV:(h + 1) * DV, :], in_=on[:])

                    # o_proj for block 3: head 0's gather lands first, so
                    # accumulate its chunks first and stop on head 1's.
                    oproj(2)
                    qsl = slice(3 * 512, 4 * 512)
                    octx = fpool.tile([128, 16, 512], BF16, tag="octx",
                                      name="octx3")
                    for hh in range(2):
                        for g in range(2):
                            q = [nc.sync, nc.gpsimd][g]
                            q.dma_start(
                                out=octx[:, hh * 8 + 4 * g: hh * 8 + 4 * (g + 1), :],
                                in_=ag3h_out[hh][g * 512:(g + 1) * 512, :]
                                    .rearrange("(c p) q -> p c q", p=128))
                    # octx slot (hh*8 + r) holds head (2r + hh) of the context
                    for m in range(2):
                        ps = bpsum.tile([128, 512], F32, tag="fps", bufs=2,
                                        name="fps3")
                        order = []
                        for hh in range(2):
                            for r in range(NC):
                                order.append((hh * 8 + r, 2 * r + hh))
                        n = len(order)
                        for idx, (slot, c) in enumerate(order):
                            nc.tensor.matmul(
                                ps[:], wo[:, c, m * 128:(m + 1) * 128],
                                octx[:, slot, :], start=(idx == 0),
                                stop=(idx == n - 1))
                        fo = fpool.tile([128, 512], F32, tag="fo", name="fo3")
                        nc.vector.tensor_copy(fo[:], ps[:])
                        nc.gpsimd.dma_start(out=d_out[m * 128:(m + 1) * 128, qsl],
                                            in_=fo[:])

    nc.finalize()
    _BUILD_CACHE["nc"] = nc
    return nc


def _prep_inputs(hidden_states, cos, sin, w_qa, w_qa_ln, w_qb, w_kva, w_kva_ln,
                 w_kvb, w_o):
    """Host-side shard + transpose + fold.  Returns per-core in_maps."""
    f32 = np.float32
    X = np.asarray(hidden_states, f32).reshape(S, HID)
    XT = np.ascontiguousarray(X.T)                      # [HID, S]
    w_qa = np.asarray(w_qa, f32)
    w_qb = np.asarray(w_qb, f32)
    w_kva = np.asarray(w_kva, f32)
    w_kvb = np.asarray(w_kvb, f32)
    w_o = np.asarray(w_o, f32)
    w_qa_ln = np.asarray(w_qa_ln, f32)
    w_kva_ln = np.asarray(w_kva_ln, f32)
    cos = np.asarray(cos, f32).reshape(S, DR)
    sin = np.asarray(sin, f32).reshape(S, DR)

    wqaT = np.ascontiguousarray(w_qa.T).astype(BF)       # [HID, QL]

    wkvaT = w_kva.T                                      # [HID, KL+DR]
    rot = wkvaT[:, KL:][:, _ROT_PERM]                    # deinterleaved
    wkvaT_ext = np.ascontiguousarray(
        np.concatenate([wkvaT[:, :KL], rot], axis=1)).astype(BF)

    # NOTE: SCALING is applied inside the on-device exp, not here.
    wqbT = (w_qb * w_qa_ln[None, :]).T                   # [QL, NH*DH]
    wqbT = wqbT.reshape(QL, NH, DH)
    qb_pass = wqbT[:, :, :DN]
    qb_rot = wqbT[:, :, DN:][:, :, _ROT_PERM]

    wkvbT = (w_kvb * w_kva_ln[None, :]).T                # [KL, NH*(DN+DV)]
    wkvbT = wkvbT.reshape(KL, NH, DN + DV)
    kvb_k = wkvbT[:, :, :DN]
    kvb_v = wkvbT[:, :, DN:]

    woT = np.ascontiguousarray(w_o.T).astype(BF)         # [NH*DV, HID]

    cosT = np.ascontiguousarray(cos.T)                   # [DR, S]
    sinT = np.ascontiguousarray(sin.T)
    sin_signed = sinT.copy()
    sin_signed[:DR // 2] *= -1.0
    cosQ = np.ascontiguousarray(np.concatenate([cosT, cosT], axis=0)).astype(BF)
    sinQ = np.ascontiguousarray(
        np.concatenate([sin_signed, sin_signed], axis=0)).astype(BF)

    # rotate-half permutation: out[i] = in[(i+32)%64] within each 64 block
    permM = np.zeros((128, 128), np.float32)
    for blk in range(2):
        for i in range(64):
            permM[blk * 64 + (i + 32) % 64, blk * 64 + i] = 1.0
    permM = permM.astype(BF)

    in_maps = []
    for i in range(NC):
        h0, h1 = HL * i, HL * i + 1
        wqbT_i = np.concatenate([
            qb_pass[:, h0], qb_pass[:, h1],
            qb_rot[:, h0], qb_rot[:, h1]], axis=1).astype(BF)
        wkvbT_i = np.concatenate([
            kvb_k[:, h0], kvb_k[:, h1],
            kvb_v[:, h0], kvb_v[:, h1]], axis=1).astype(BF)
        sls = slice(SL * i, SL * (i + 1))
        in_maps.append(dict(
            xT=np.ascontiguousarray(XT[:, sls]).astype(BF),
            wqaT=wqaT,
            wkvaT=wkvaT_ext,
            wqbT=np.ascontiguousarray(wqbT_i),
            wkvbT=np.ascontiguousarray(wkvbT_i),
            woT=np.ascontiguousarray(woT[:, HIDL * i: HIDL * (i + 1)]),
            cosS=np.ascontiguousarray(cosT[:, sls]),
            sinS=np.ascontiguousarray(sin_signed[:, sls]),
            cosQ=cosQ,
            sinQ=sinQ,
            permM=permM,
        ))
    return in_maps


def kernel(**inputs):
    nc = _build_nc()
    in_maps = _prep_inputs(**inputs)
    res = run_bass_kernel_spmd(nc, in_maps, core_ids=list(range(NC)))
    finalT = np.concatenate([res.results[i]["o"] for i in range(NC)], axis=0)
    out = np.ascontiguousarray(finalT.T).reshape(B, S, HID).astype(np.float32)
    return out
